# revision 25
# baseline (speedup 1.0000x reference)
"""Causal self-attention Trainium2 kernel.

B=2, T=2048, C=1024, H=16, D=64. 8 NeuronCores: core = b*4 + head_group,
data parallel over batch (b = core//4), tensor parallel over heads
(4 heads per core). Each core computes its heads' qkv projection,
causal+key-masked attention, and a partial output projection over its
256 input channels; the host sums the per-core/per-head-pair partials
per batch element and adds the proj bias.

All on-device layouts are transposed so softmax works per-partition:
  xT   [C, T]      q^T/k^T [2*64, T] per head-pair (partition = head dim)
  s^T  [k, q]      exp bias (per-partition = k) applies the key padding mask
  out^T[d, q]      col-tiled p@v; directly the lhsT of the proj matmul
The softmax denominator l is produced by an ones-lhsT matmul that
broadcasts l across each head's 64 partitions, so normalization is one
reciprocal + one multiply.
"""

import sys

sys.path.insert(0, "/opt/trn_rl_repo")

import numpy as np
import ml_dtypes

import concourse.bass as bass
import concourse.mybir as mybir
import concourse.tile as tile
from concourse import bacc
from concourse.bass import ts, ds
from concourse.bass_utils import run_bass_kernel_spmd

B, T, C, H = 2, 2048, 1024, 16
D = C // H            # 64
HPC = 4               # heads per core
CS = HPC * D          # 256 channel slice per core
NCORE = 8
NKT = T // 128        # 16 k-tiles
NCH = T // 512        # 4 q-chunks
NCT = C // 128        # 8 contraction tiles
F32 = mybir.dt.float32
F32R = mybir.dt.float32r
BF16 = mybir.dt.bfloat16
EXP = mybir.ActivationFunctionType.Exp

NEG = -30000.0


def build_nc():
    nc = bacc.Bacc("TRN2", target_bir_lowering=False, debug=False,
                   num_devices=NCORE)

    xT = nc.dram_tensor("xT", [C, T], BF16, kind="ExternalInput")
    wqT = nc.dram_tensor("wqT", [C, CS], BF16, kind="ExternalInput")
    wkT = nc.dram_tensor("wkT", [C, CS], BF16, kind="ExternalInput")
    wvT = nc.dram_tensor("wvT", [C, CS], BF16, kind="ExternalInput")
    wpT = nc.dram_tensor("wpT", [CS, C], BF16, kind="ExternalInput")
    bq = nc.dram_tensor("bq", [128, 2], F32, kind="ExternalInput")
    bk = nc.dram_tensor("bk", [128, 2], F32, kind="ExternalInput")
    vbias = nc.dram_tensor("vbias", [128, CS], F32, kind="ExternalInput")
    kbias = nc.dram_tensor("kbias", [128, NKT], F32, kind="ExternalInput")
    tri = nc.dram_tensor("tri", [128, 512], BF16, kind="ExternalInput")
    ones = nc.dram_tensor("ones", [128, 64], BF16, kind="ExternalInput")
    y = nc.dram_tensor("y", [T, C], BF16, kind="ExternalOutput")

    with tile.TileContext(nc) as tc:
        with (
            tc.tile_pool(name="const", bufs=1) as const,
            tc.tile_pool(name="acts", bufs=1) as acts,
            tc.tile_pool(name="p", bufs=8) as ppool,
            tc.tile_pool(name="ev", bufs=4) as ev,
            tc.tile_pool(name="psum", bufs=1, space="PSUM") as psum,
        ):
            # ---- input DMAs, ordered by first use ----
            wq_t = [const.tile([128, CS], BF16, tag=f"wq{i}", name=f"wq{i}")
                    for i in range(NCT)]
            x_t = [const.tile([128, T], BF16, tag=f"x{i}", name=f"x{i}")
                   for i in range(NCT)]
            wk_t = [const.tile([128, CS], BF16, tag=f"wk{i}", name=f"wk{i}")
                    for i in range(NCT)]
            wv_t = [const.tile([128, CS], BF16, tag=f"wv{i}", name=f"wv{i}")
                    for i in range(NCT)]
            bq_t = const.tile([128, 2], F32, tag="bq")
            bk_t = const.tile([128, 2], F32, tag="bk")
            vb_t = const.tile([128, CS], F32, tag="vb")
            kb_t = const.tile([128, NKT], F32, tag="kb")
            tri_t = const.tile([128, 512], BF16, tag="tri")
            on_t = const.tile([128, 64], BF16, tag="ones")
            wp_t = [const.tile([128, C], BF16, tag=f"wp{i}", name=f"wp{i}")
                    for i in range(2)]
            # DMAs are emitted interleaved with the first compute waves so
            # queue order matches consumption order (see below). Only the
            # small bias/const tensors are loaded up front.
            nc.sync.dma_start(bq_t[:], bq[:])
            nc.sync.dma_start(bk_t[:], bk[:])

            # ---- phase 1: q^T/k^T/v; hp1's q/k and most of v are deferred
            # into attention-hp0's PE stream via generators ----
            qT = [acts.tile([128, T], BF16, tag=f"qT{hp}", name=f"qT{hp}")
                  for hp in range(2)]
            kT = [acts.tile([128, T], BF16, tag=f"kT{hp}", name=f"kT{hp}")
                  for hp in range(2)]
            v_t = [acts.tile([128, CS], BF16, tag=f"v{tt}", name=f"v{tt}")
                   for tt in range(NKT)]

            def qk_chunk_gen(w_t, b_t, dst, ot, tch):
                ps = psum.tile([128, 512], F32, tag="pj", name="ps_qk",
                               bufs=2)
                for ct in range(NCT):
                    nc.tensor.matmul(
                        ps[:],
                        w_t[ct][:, ts(ot, 128)],
                        x_t[ct][:, ts(tch, 512)],
                        start=(ct == 0), stop=(ct == NCT - 1),
                        skip_group_check=True,
                    )
                    yield
                nc.vector.tensor_scalar_add(
                    dst[ot][:, ts(tch, 512)], ps[:], b_t[:, ot:ot + 1])
                yield

            def v_chunk_gen(tt):
                ps = psum.tile([128, CS], F32, tag="pj", name="ps_v", bufs=2)
                for ct in range(NCT):
                    nc.tensor.matmul(
                        ps[:],
                        x_t[ct][:, ts(tt, 128)],
                        wv_t[ct][:],
                        start=(ct == 0), stop=(ct == NCT - 1),
                        skip_group_check=True,
                    )
                    yield
                nc.vector.tensor_add(v_t[tt][:], ps[:], vb_t[:])
                yield

            def drain_gen(g):
                for _ in g:
                    pass

            # PE warm-up: dummy matmuls on a memset tile keep the HAM clock
            # warm while the input DMAs stream in
            warm = const.tile([128, 640], BF16, tag="warm")
            nc.vector.memset(warm[:], 0.0)
            wps = psum.tile([128, 512], F32, tag="pj", name="ps_warm", bufs=2)
            for i in range(6):
                nc.tensor.matmul(wps[:], warm[:, 0:128], warm[:, 128:640],
                                 start=True, stop=True, skip_group_check=True)

            # hp0's q wave 0 (tch 0,1) with per-ct DMA emission; tch 2,3
            # are deferred into attention-hp0's gap-filler stream
            for wave in range(1):
                slots = [psum.tile([128, 512], F32, tag="s",
                                   name=f"ps_q0{half}", bufs=4)
                         for half in range(2)]
                for ct in range(NCT):
                    nc.sync.dma_start(wq_t[ct][:], wqT[ts(ct, 128), :])
                    nc.sync.dma_start(x_t[ct][:], xT[ts(ct, 128), :])
                    for half in range(2):
                        tch = 2 * wave + half
                        nc.tensor.matmul(
                            slots[half][:],
                            wq_t[ct][:, ts(0, 128)],
                            x_t[ct][:, ts(tch, 512)],
                            start=(ct == 0), stop=(ct == NCT - 1),
                            skip_group_check=True,
                        )
                for half in range(2):
                    tch = 2 * wave + half
                    nc.vector.tensor_scalar_add(
                        qT[0][:, ts(tch, 512)], slots[half][:],
                        bq_t[:, 0:1])
            # hp0's k waves, wk DMA per ct
            for wave in range(1):
                slots = [psum.tile([128, 512], F32, tag="s",
                                   name=f"ps_k0{half}", bufs=4)
                         for half in range(2)]
                for ct in range(NCT):
                    nc.sync.dma_start(wk_t[ct][:], wkT[ts(ct, 128), :])
                    for half in range(2):
                        tch = 2 * wave + half
                        nc.tensor.matmul(
                            slots[half][:],
                            wk_t[ct][:, ts(0, 128)],
                            x_t[ct][:, ts(tch, 512)],
                            start=(ct == 0), stop=(ct == NCT - 1),
                            skip_group_check=True,
                        )
                for half in range(2):
                    tch = 2 * wave + half
                    nc.vector.tensor_scalar_add(
                        kT[0][:, ts(tch, 512)], slots[half][:],
                        bk_t[:, 0:1])
            for i in range(NCT):
                nc.sync.dma_start(wv_t[i][:], wvT[ts(i, 128), :])
            nc.sync.dma_start(vb_t[:], vbias[:])
            nc.sync.dma_start(kb_t[:], kbias[:])
            nc.sync.dma_start(tri_t[:], tri[:])
            nc.sync.dma_start(on_t[:], ones[:])
            for i in range(2):
                nc.sync.dma_start(wp_t[i][:], wpT[ts(i, 128), :])
            for tt in range(6):
                drain_gen(v_chunk_gen(tt))

            # deferred (deadline order): hp0's q/k tch2, v6-7, tch3, the
            # rest of v, then hp1's q/k chunks
            deferred = [
                qk_chunk_gen(wq_t, bq_t, qT, 0, 2),
                qk_chunk_gen(wk_t, bk_t, kT, 0, 2),
                v_chunk_gen(6),
                v_chunk_gen(7),
                qk_chunk_gen(wq_t, bq_t, qT, 0, 3),
                qk_chunk_gen(wk_t, bk_t, kT, 0, 3),
            ]
            for tt in range(8, NKT):
                deferred.append(v_chunk_gen(tt))
            for w_t, b_t, dst in ((wq_t, bq_t, qT), (wk_t, bk_t, kT)):
                for tch in range(NCH):
                    deferred.append(qk_chunk_gen(w_t, b_t, dst, 1, tch))

            # ---- phase 2+3: attention per head-pair, then its proj ----
            attnT = [acts.tile([128, T], BF16, tag=f"aT{hp}", name=f"aT{hp}")
                     for hp in range(2)]

            def scores_exp(hp, ch, kt, h, pend):
                q_lo = 512 * ch
                diag = (kt // 4 == ch)
                off = 128 * (kt % 4) if diag else 0
                n = 512 - off
                qs = q_lo + off
                ps_s = psum.tile([128, 512], F32, tag="s", name="ps_s",
                                 bufs=4)
                nc.tensor.matmul(
                    ps_s[:, ds(off, n)],
                    kT[hp][ds(64 * h, 64), ts(kt, 128)],
                    qT[hp][ds(64 * h, 64), ds(qs, n)],
                    start=True, stop=True,
                )
                p = ppool.tile([128, 512], BF16, tag="p", name="p")
                nc.scalar.activation(
                    p[:, ds(off, n)], ps_s[:, ds(off, n)], EXP,
                    bias=kb_t[:, kt:kt + 1], scale=0.125)
                if diag:
                    blk = p[:, ds(off, 128)]
                    nc.vector.tensor_mul(blk, blk, tri_t[:, ds(384, 128)])
                pend.append((kt, h, off, n, p))

            def pv_l2(hp, ent0, ent1, ps_o, ps_l, nkt):
                # both heads' pv (col groups 0-1 / 2-3) back-to-back so they
                # run concurrently in the array, then both l matmuls
                for h, (kt, _h, off, n, p) in enumerate((ent0, ent1)):
                    nc.tensor.matmul(
                        ps_o[ds(64 * h, 64), ds(off, n)],
                        v_t[kt][:, ds(128 * hp + 64 * h, 64)],
                        p[:, ds(off, n)],
                        start=(kt == 0), stop=(kt == nkt - 1),
                        skip_group_check=True,
                    )
                for h, (kt, _h, off, n, p) in enumerate((ent0, ent1)):
                    nc.tensor.matmul(
                        ps_l[ds(64 * h, 64), ds(off, n)],
                        on_t[:, 0:64],
                        p[:, ds(off, n)],
                        start=(kt == 0), stop=(kt == nkt - 1),
                        skip_group_check=True,
                    )

            nev = [0]

            def proj_tile(tt, cch, use_act=False):
                ps = psum.tile([128, 512], F32, tag="pj",
                               name="ps_y", bufs=2)
                for hp in range(2):
                    nc.tensor.matmul(
                        ps[:],
                        attnT[hp][:, ts(tt, 128)],
                        wp_t[hp][:, ts(cch, 512)],
                        start=(hp == 0), stop=(hp == 1),
                        skip_group_check=True,
                    )
                ysb = ev.tile([128, 512], BF16, tag="y", name="ysb")
                if use_act:
                    nc.scalar.copy(ysb[:], ps[:])
                else:
                    nc.vector.tensor_copy(ysb[:], ps[:])
                nev[0] += 1
                nc.sync.dma_start(y[ts(tt, 128), ts(cch, 512)], ysb[:])

            proj_q = []
            for hp in range(2):
                for ch in range(NCH):
                    q_lo = 512 * ch
                    nkt = 4 * (ch + 1)
                    ps_o = psum.tile([128, 512], F32, tag="o", name="ps_o")
                    ps_l = psum.tile([128, 512], F32, tag="l", name="ps_l")
                    pend = []
                    for kt2 in range(0, nkt, 2):
                        # 4-MM score burst: row groups alternate 0/64 so
                        # LDWEIGHTS chains pull ahead of in-flight matmuls
                        scores_exp(hp, ch, kt2, 0, pend)
                        scores_exp(hp, ch, kt2, 1, pend)
                        scores_exp(hp, ch, kt2 + 1, 0, pend)
                        scores_exp(hp, ch, kt2 + 1, 1, pend)
                        while len(pend) > 4:
                            e0 = pend.pop(0)
                            e1 = pend.pop(0)
                            pv_l2(hp, e0, e1, ps_o, ps_l, nkt)
                        # drive deferred qkv work (hp0) / proj (hp1)
                        steps = 8
                        while steps > 0 and deferred:
                            try:
                                next(deferred[0])
                                steps -= 1
                            except StopIteration:
                                deferred.pop(0)
                        for _ in range(2):
                            if proj_q:
                                proj_tile(*proj_q.pop(0))
                    while pend:
                        e0 = pend.pop(0)
                        e1 = pend.pop(0)
                        pv_l2(hp, e0, e1, ps_o, ps_l, nkt)
                    # normalize: attn^T = out^T / l
                    rec = ev.tile([128, 512], F32, tag="rec", name="rec")
                    nc.vector.reciprocal_approx_fast(rec[:], ps_l[:])
                    nc.vector.tensor_mul(
                        attnT[hp][:, ds(q_lo, 512)], ps_o[:], rec[:])
                    # proj needs both head-pairs' attnT: queue during hp1
                    if hp == 1:
                        for tt in range(4 * ch, 4 * ch + 4):
                            for cch in range(2):
                                proj_q.append((tt, cch))
                if hp == 0:
                    while deferred:
                        drain_gen(deferred.pop(0))
            for i in range(len(proj_q)):
                proj_tile(*proj_q.pop(0), use_act=(i % 2 == 1))

    nc.compile()
    return nc


def shard_inputs(x, key_padding_mask, Wqkv, bqkv, Wproj, bproj):
    bf = ml_dtypes.bfloat16
    tri = np.zeros((128, 512), bf)
    tri[:, 384:] = (np.arange(128)[:, None] <= np.arange(128)[None, :])
    ones = np.ones((128, 64), bf)
    in_maps = []
    for core in range(NCORE):
        b, g = core // HPC, core % HPC
        qs = slice(CS * g, CS * g + CS)
        ks = slice(C + CS * g, C + CS * g + CS)
        vs = slice(2 * C + CS * g, 2 * C + CS * g + CS)
        kb = np.where(key_padding_mask[b], 0.0, NEG).astype(np.float32)
        in_maps.append({
            "xT": np.ascontiguousarray(x[b].T).astype(bf),
            "wqT": np.ascontiguousarray(Wqkv[qs].T).astype(bf),
            "wkT": np.ascontiguousarray(Wqkv[ks].T).astype(bf),
            "wvT": np.ascontiguousarray(Wqkv[vs].T).astype(bf),
            "wpT": np.ascontiguousarray(Wproj[:, CS * g:CS * g + CS].T).astype(bf),
            "bq": np.ascontiguousarray(bqkv[qs].reshape(2, 128).T),
            "bk": np.ascontiguousarray(bqkv[ks].reshape(2, 128).T),
            "vbias": np.ascontiguousarray(
                np.broadcast_to(bqkv[vs], (128, CS))),
            "kbias": np.ascontiguousarray(kb.reshape(NKT, 128).T),
            "tri": tri,
            "ones": ones,
        })
    return in_maps


_NC_CACHE = None


def kernel(x, key_padding_mask, Wqkv, bqkv, Wproj, bproj):
    global _NC_CACHE
    if _NC_CACHE is None:
        _NC_CACHE = build_nc()
    nc = _NC_CACHE
    in_maps = shard_inputs(x, key_padding_mask, Wqkv, bqkv, Wproj, bproj)
    res = run_bass_kernel_spmd(nc, in_maps, list(range(NCORE)))
    if not all(np.isfinite(np.asarray(r["y"], dtype=np.float32)).all()
               for r in res.results):
        # very rare first-execution flake: retry once
        res = run_bass_kernel_spmd(nc, in_maps, list(range(NCORE)))
    out = np.empty((B, T, C), np.float32)
    for b in range(B):
        acc = np.zeros((T, C), np.float64)
        for g in range(HPC):
            acc += np.asarray(res.results[4 * b + g]["y"], dtype=np.float64)
        out[b] = (acc + np.asarray(bproj)).astype(np.float32)
    return out


# revision 26
# speedup vs baseline: 1.0270x; 1.0270x over previous
"""Causal self-attention Trainium2 kernel.

B=2, T=2048, C=1024, H=16, D=64. 8 NeuronCores: core = b*4 + head_group,
data parallel over batch (b = core//4), tensor parallel over heads
(4 heads per core). Each core computes its heads' qkv projection,
causal+key-masked attention, and a partial output projection over its
256 input channels; the host sums the per-core/per-head-pair partials
per batch element and adds the proj bias.

All on-device layouts are transposed so softmax works per-partition:
  xT   [C, T]      q^T/k^T [2*64, T] per head-pair (partition = head dim)
  s^T  [k, q]      exp bias (per-partition = k) applies the key padding mask
  out^T[d, q]      col-tiled p@v; directly the lhsT of the proj matmul
The softmax denominator l is produced by an ones-lhsT matmul that
broadcasts l across each head's 64 partitions, so normalization is one
reciprocal + one multiply.
"""

import sys

sys.path.insert(0, "/opt/trn_rl_repo")

import numpy as np
import ml_dtypes

import concourse.bass as bass
import concourse.mybir as mybir
import concourse.tile as tile
from concourse import bacc
from concourse.bass import ts, ds
from concourse.bass_utils import run_bass_kernel_spmd

B, T, C, H = 2, 2048, 1024, 16
D = C // H            # 64
HPC = 4               # heads per core
CS = HPC * D          # 256 channel slice per core
NCORE = 8
NKT = T // 128        # 16 k-tiles
NCH = T // 512        # 4 q-chunks
NCT = C // 128        # 8 contraction tiles
F32 = mybir.dt.float32
F32R = mybir.dt.float32r
BF16 = mybir.dt.bfloat16
EXP = mybir.ActivationFunctionType.Exp

NEG = -30000.0


def build_nc():
    nc = bacc.Bacc("TRN2", target_bir_lowering=False, debug=False,
                   num_devices=NCORE)

    xT = nc.dram_tensor("xT", [C, T], BF16, kind="ExternalInput")
    wqT = nc.dram_tensor("wqT", [C, CS], BF16, kind="ExternalInput")
    wkT = nc.dram_tensor("wkT", [C, CS], BF16, kind="ExternalInput")
    wvT = nc.dram_tensor("wvT", [C, CS], BF16, kind="ExternalInput")
    wpT = nc.dram_tensor("wpT", [CS, C], BF16, kind="ExternalInput")
    bq = nc.dram_tensor("bq", [128, 2], F32, kind="ExternalInput")
    bk = nc.dram_tensor("bk", [128, 2], F32, kind="ExternalInput")
    vbias = nc.dram_tensor("vbias", [128, CS], F32, kind="ExternalInput")
    kbias = nc.dram_tensor("kbias", [128, NKT], F32, kind="ExternalInput")
    tri = nc.dram_tensor("tri", [128, 512], BF16, kind="ExternalInput")
    ones = nc.dram_tensor("ones", [128, 64], BF16, kind="ExternalInput")
    y = nc.dram_tensor("y", [T, C], BF16, kind="ExternalOutput")

    with tile.TileContext(nc) as tc:
        with (
            tc.tile_pool(name="const", bufs=1) as const,
            tc.tile_pool(name="acts", bufs=1) as acts,
            tc.tile_pool(name="p", bufs=8) as ppool,
            tc.tile_pool(name="ev", bufs=4) as ev,
            tc.tile_pool(name="psum", bufs=1, space="PSUM") as psum,
        ):
            # ---- input DMAs, ordered by first use ----
            wq_t = [const.tile([128, CS], BF16, tag=f"wq{i}", name=f"wq{i}")
                    for i in range(NCT)]
            x_t = [const.tile([128, T], BF16, tag=f"x{i}", name=f"x{i}")
                   for i in range(NCT)]
            wk_t = [const.tile([128, CS], BF16, tag=f"wk{i}", name=f"wk{i}")
                    for i in range(NCT)]
            wv_t = [const.tile([128, CS], BF16, tag=f"wv{i}", name=f"wv{i}")
                    for i in range(NCT)]
            bq_t = const.tile([128, 2], F32, tag="bq")
            bk_t = const.tile([128, 2], F32, tag="bk")
            vb_t = const.tile([128, CS], F32, tag="vb")
            kb_t = const.tile([128, NKT], F32, tag="kb")
            tri_t = const.tile([128, 512], BF16, tag="tri")
            on_t = const.tile([128, 64], BF16, tag="ones")
            wp_t = [const.tile([128, C], BF16, tag=f"wp{i}", name=f"wp{i}")
                    for i in range(2)]
            # DMAs are emitted interleaved with the first compute waves so
            # queue order matches consumption order (see below). Only the
            # small bias/const tensors are loaded up front.
            nc.sync.dma_start(bq_t[:], bq[:])
            nc.sync.dma_start(bk_t[:], bk[:])

            # ---- phase 1: q^T/k^T/v; hp1's q/k and most of v are deferred
            # into attention-hp0's PE stream via generators ----
            qT = [acts.tile([128, T], BF16, tag=f"qT{hp}", name=f"qT{hp}")
                  for hp in range(2)]
            kT = [acts.tile([128, T], BF16, tag=f"kT{hp}", name=f"kT{hp}")
                  for hp in range(2)]
            v_t = [acts.tile([128, CS], BF16, tag=f"v{tt}", name=f"v{tt}")
                   for tt in range(NKT)]

            def qk_chunk_gen(w_t, b_t, dst, ot, tch):
                ps = psum.tile([128, 512], F32, tag="pj", name="ps_qk",
                               bufs=2)
                for ct in range(NCT):
                    nc.tensor.matmul(
                        ps[:],
                        w_t[ct][:, ts(ot, 128)],
                        x_t[ct][:, ts(tch, 512)],
                        start=(ct == 0), stop=(ct == NCT - 1),
                        skip_group_check=True,
                    )
                    yield
                nc.vector.tensor_scalar_add(
                    dst[ot][:, ts(tch, 512)], ps[:], b_t[:, ot:ot + 1])
                yield

            def v_chunk_gen(tt):
                ps = psum.tile([128, CS], F32, tag="pj", name="ps_v", bufs=2)
                for ct in range(NCT):
                    nc.tensor.matmul(
                        ps[:],
                        x_t[ct][:, ts(tt, 128)],
                        wv_t[ct][:],
                        start=(ct == 0), stop=(ct == NCT - 1),
                        skip_group_check=True,
                    )
                    yield
                nc.vector.tensor_add(v_t[tt][:], ps[:], vb_t[:])
                yield

            def drain_gen(g):
                for _ in g:
                    pass

            # PE warm-up: dummy matmuls on a memset tile keep the HAM clock
            # warm while the input DMAs stream in
            warm = const.tile([128, 640], BF16, tag="warm")
            nc.vector.memset(warm[:], 0.0)
            wps = psum.tile([128, 512], F32, tag="pj", name="ps_warm", bufs=2)
            for i in range(6):
                nc.tensor.matmul(wps[:], warm[:, 0:128], warm[:, 128:640],
                                 start=True, stop=True, skip_group_check=True)

            # hp0's q waves with per-ct DMA emission (queue order = use order)
            for wave in range(2):
                slots = [psum.tile([128, 512], F32, tag="s",
                                   name=f"ps_q0{half}", bufs=4)
                         for half in range(2)]
                for ct in range(NCT):
                    if wave == 0:
                        nc.sync.dma_start(wq_t[ct][:], wqT[ts(ct, 128), :])
                        nc.sync.dma_start(x_t[ct][:], xT[ts(ct, 128), :])
                    for half in range(2):
                        tch = 2 * wave + half
                        nc.tensor.matmul(
                            slots[half][:],
                            wq_t[ct][:, ts(0, 128)],
                            x_t[ct][:, ts(tch, 512)],
                            start=(ct == 0), stop=(ct == NCT - 1),
                            skip_group_check=True,
                        )
                for half in range(2):
                    tch = 2 * wave + half
                    nc.vector.tensor_scalar_add(
                        qT[0][:, ts(tch, 512)], slots[half][:],
                        bq_t[:, 0:1])
            # hp0's k waves, wk DMA per ct
            for wave in range(2):
                slots = [psum.tile([128, 512], F32, tag="s",
                                   name=f"ps_k0{half}", bufs=4)
                         for half in range(2)]
                for ct in range(NCT):
                    if wave == 0:
                        nc.sync.dma_start(wk_t[ct][:], wkT[ts(ct, 128), :])
                    for half in range(2):
                        tch = 2 * wave + half
                        nc.tensor.matmul(
                            slots[half][:],
                            wk_t[ct][:, ts(0, 128)],
                            x_t[ct][:, ts(tch, 512)],
                            start=(ct == 0), stop=(ct == NCT - 1),
                            skip_group_check=True,
                        )
                for half in range(2):
                    tch = 2 * wave + half
                    nc.vector.tensor_scalar_add(
                        kT[0][:, ts(tch, 512)], slots[half][:],
                        bk_t[:, 0:1])
            for i in range(NCT):
                nc.sync.dma_start(wv_t[i][:], wvT[ts(i, 128), :])
            nc.sync.dma_start(vb_t[:], vbias[:])
            nc.sync.dma_start(kb_t[:], kbias[:])
            nc.sync.dma_start(tri_t[:], tri[:])
            nc.sync.dma_start(on_t[:], ones[:])
            for i in range(2):
                nc.sync.dma_start(wp_t[i][:], wpT[ts(i, 128), :])
            for tt in range(7):
                drain_gen(v_chunk_gen(tt))

            # deferred: v[7..15], then hp1's q/k chunks
            deferred = []
            for tt in range(7, NKT):
                deferred.append(v_chunk_gen(tt))
            for w_t, b_t, dst in ((wq_t, bq_t, qT), (wk_t, bk_t, kT)):
                for tch in range(NCH):
                    deferred.append(qk_chunk_gen(w_t, b_t, dst, 1, tch))

            # ---- phase 2+3: attention per head-pair, then its proj ----
            attnT = [acts.tile([128, T], BF16, tag=f"aT{hp}", name=f"aT{hp}")
                     for hp in range(2)]

            def scores_exp(hp, ch, kt, h, pend):
                q_lo = 512 * ch
                diag = (kt // 4 == ch)
                off = 128 * (kt % 4) if diag else 0
                n = 512 - off
                qs = q_lo + off
                ps_s = psum.tile([128, 512], F32, tag="s", name="ps_s",
                                 bufs=4)
                nc.tensor.matmul(
                    ps_s[:, ds(off, n)],
                    kT[hp][ds(64 * h, 64), ts(kt, 128)],
                    qT[hp][ds(64 * h, 64), ds(qs, n)],
                    start=True, stop=True,
                )
                p = ppool.tile([128, 512], BF16, tag="p", name="p")
                nc.scalar.activation(
                    p[:, ds(off, n)], ps_s[:, ds(off, n)], EXP,
                    bias=kb_t[:, kt:kt + 1], scale=0.125)
                if diag:
                    blk = p[:, ds(off, 128)]
                    nc.vector.tensor_mul(blk, blk, tri_t[:, ds(384, 128)])
                pend.append((kt, h, off, n, p))

            def pv_l2(hp, ent0, ent1, ps_o, ps_l, nkt):
                # both heads' pv (col groups 0-1 / 2-3) back-to-back so they
                # run concurrently in the array, then both l matmuls
                for h, (kt, _h, off, n, p) in enumerate((ent0, ent1)):
                    nc.tensor.matmul(
                        ps_o[ds(64 * h, 64), ds(off, n)],
                        v_t[kt][:, ds(128 * hp + 64 * h, 64)],
                        p[:, ds(off, n)],
                        start=(kt == 0), stop=(kt == nkt - 1),
                        skip_group_check=True,
                    )
                for h, (kt, _h, off, n, p) in enumerate((ent0, ent1)):
                    nc.tensor.matmul(
                        ps_l[ds(64 * h, 64), ds(off, n)],
                        on_t[:, 0:64],
                        p[:, ds(off, n)],
                        start=(kt == 0), stop=(kt == nkt - 1),
                        skip_group_check=True,
                    )

            nev = [0]

            def proj_tile(tt, cch, use_act=False):
                ps = psum.tile([128, 512], F32, tag="pj",
                               name="ps_y", bufs=2)
                for hp in range(2):
                    nc.tensor.matmul(
                        ps[:],
                        attnT[hp][:, ts(tt, 128)],
                        wp_t[hp][:, ts(cch, 512)],
                        start=(hp == 0), stop=(hp == 1),
                        skip_group_check=True,
                    )
                ysb = ev.tile([128, 512], BF16, tag="y", name="ysb")
                if use_act:
                    nc.scalar.copy(ysb[:], ps[:])
                else:
                    nc.vector.tensor_copy(ysb[:], ps[:])
                nev[0] += 1
                nc.sync.dma_start(y[ts(tt, 128), ts(cch, 512)], ysb[:])

            proj_q = []
            for hp in range(2):
                for ch in range(NCH):
                    q_lo = 512 * ch
                    nkt = 4 * (ch + 1)
                    ps_o = psum.tile([128, 512], F32, tag="o", name="ps_o")
                    ps_l = psum.tile([128, 512], F32, tag="l", name="ps_l")
                    pend = []
                    for kt2 in range(0, nkt, 2):
                        # 4-MM score burst: row groups alternate 0/64 so
                        # LDWEIGHTS chains pull ahead of in-flight matmuls
                        scores_exp(hp, ch, kt2, 0, pend)
                        scores_exp(hp, ch, kt2, 1, pend)
                        scores_exp(hp, ch, kt2 + 1, 0, pend)
                        scores_exp(hp, ch, kt2 + 1, 1, pend)
                        while len(pend) > 4:
                            e0 = pend.pop(0)
                            e1 = pend.pop(0)
                            pv_l2(hp, e0, e1, ps_o, ps_l, nkt)
                        # drive deferred qkv work (hp0) / proj (hp1)
                        steps = 8
                        while steps > 0 and deferred:
                            try:
                                next(deferred[0])
                                steps -= 1
                            except StopIteration:
                                deferred.pop(0)
                        for _ in range(2):
                            if proj_q:
                                proj_tile(*proj_q.pop(0))
                    while pend:
                        e0 = pend.pop(0)
                        e1 = pend.pop(0)
                        pv_l2(hp, e0, e1, ps_o, ps_l, nkt)
                    # normalize: attn^T = out^T / l
                    rec = ev.tile([128, 512], F32, tag="rec", name="rec")
                    nc.vector.reciprocal_approx_fast(rec[:], ps_l[:])
                    nc.vector.tensor_mul(
                        attnT[hp][:, ds(q_lo, 512)], ps_o[:], rec[:])
                    # proj needs both head-pairs' attnT: queue during hp1
                    if hp == 1:
                        for tt in range(4 * ch, 4 * ch + 4):
                            for cch in range(2):
                                proj_q.append((tt, cch))
                if hp == 0:
                    while deferred:
                        drain_gen(deferred.pop(0))
            for i in range(len(proj_q)):
                proj_tile(*proj_q.pop(0), use_act=(i % 2 == 1))

    nc.compile()
    return nc


def shard_inputs(x, key_padding_mask, Wqkv, bqkv, Wproj, bproj):
    bf = ml_dtypes.bfloat16
    tri = np.zeros((128, 512), bf)
    tri[:, 384:] = (np.arange(128)[:, None] <= np.arange(128)[None, :])
    ones = np.ones((128, 64), bf)
    in_maps = []
    for core in range(NCORE):
        b, g = core // HPC, core % HPC
        qs = slice(CS * g, CS * g + CS)
        ks = slice(C + CS * g, C + CS * g + CS)
        vs = slice(2 * C + CS * g, 2 * C + CS * g + CS)
        kb = np.where(key_padding_mask[b], 0.0, NEG).astype(np.float32)
        in_maps.append({
            "xT": np.ascontiguousarray(x[b].T).astype(bf),
            "wqT": np.ascontiguousarray(Wqkv[qs].T).astype(bf),
            "wkT": np.ascontiguousarray(Wqkv[ks].T).astype(bf),
            "wvT": np.ascontiguousarray(Wqkv[vs].T).astype(bf),
            "wpT": np.ascontiguousarray(Wproj[:, CS * g:CS * g + CS].T).astype(bf),
            "bq": np.ascontiguousarray(bqkv[qs].reshape(2, 128).T),
            "bk": np.ascontiguousarray(bqkv[ks].reshape(2, 128).T),
            "vbias": np.ascontiguousarray(
                np.broadcast_to(bqkv[vs], (128, CS))),
            "kbias": np.ascontiguousarray(kb.reshape(NKT, 128).T),
            "tri": tri,
            "ones": ones,
        })
    return in_maps


_NC_CACHE = None


def kernel(x, key_padding_mask, Wqkv, bqkv, Wproj, bproj):
    global _NC_CACHE
    if _NC_CACHE is None:
        _NC_CACHE = build_nc()
    nc = _NC_CACHE
    in_maps = shard_inputs(x, key_padding_mask, Wqkv, bqkv, Wproj, bproj)
    res = run_bass_kernel_spmd(nc, in_maps, list(range(NCORE)))
    if not all(np.isfinite(np.asarray(r["y"], dtype=np.float32)).all()
               for r in res.results):
        # very rare first-execution flake: retry once
        res = run_bass_kernel_spmd(nc, in_maps, list(range(NCORE)))
    out = np.empty((B, T, C), np.float32)
    for b in range(B):
        acc = np.zeros((T, C), np.float64)
        for g in range(HPC):
            acc += np.asarray(res.results[4 * b + g]["y"], dtype=np.float64)
        out[b] = (acc + np.asarray(bproj)).astype(np.float32)
    return out
